# revision 41
# baseline (speedup 1.0000x reference)
"""AttnBlock (channel attention over 64x64 maps) for Trainium2 — factored
epilogue + transpose-Gram + stats-from-Gram edition.

Data-parallel over batch: 16 batches, 2 per core on 8 NeuronCores.
Per batch [C=512, N=4096], hn = A*x + B (GroupNorm folded to per-channel
affine):

  scores = q^T k with q = Wq hn + bq factorizes through the RAW Gram
    matrix Graw = x @ x^T (C x C):
      scores = (256 Wq A) Graw (256 A Wk)^T / 65536 + rank-1 corrections
    with diag(A) folded into per-batch scaled weights wqa/wka, so the
    pixel-major xT build is a PURE transpose that starts as soon as the
    first x chunks land. The GroupNorm stats come from the Gram itself:
    per-channel sum(x^2) is the Gram diagonal (exact: bf16*bf16 products
    accumulate exactly in fp32) and per-channel sum(x) rides along as a
    ones-column appended to the transposed tiles (blocks 1-3; block 0's
    PSUM bank is full, its rowsum comes from a gpsimd reduce). bn_stats
    and its 22us of serial vector time are gone. The group aggregation
    runs in bf16 matmuls with hi/lo operand splitting (fp32 accuracy at
    bf16 speed).
  The epilogue factors the same way: out = Wo attn v collapses to
      y = M' x + r 1^T,  M' = I + Wo attn Wv diag(A),
      r = Wo attn bvb + bo
    via R = e^T (rinv*Wo^T), M^T = Wv R (A-scaled at eviction, identity
    added), then ONE C*C*N application off the resident bf16 x, evicted
    bf16 with vector/scalar alternation and paired output DMAs spread
    over the sync and gpsimd queues.
Batches are software-pipelined; batch1's Gram covers batch0's softmax;
batch0's Y GEMM is split around batch1's t1t/scores to cover seams.
"""

import sys

if "/opt/trn_rl_repo" not in sys.path:
    sys.path.insert(0, "/opt/trn_rl_repo")

import numpy as np

C = 512          # channels
N = 4096         # pixels (64*64)
BB = 2           # batches per core
P = 128          # partitions
CB = C // P      # 4 channel blocks
NT = N // P      # 32 pixel tiles of 128
NTH = 16         # pixel tiles per hnT chunk
NSL = 512        # pixel slice width (y phase)
NS = N // NSL    # 8 pixel slices
NCH = 4          # x load chunks (batch 0)
GROUPS = 32
EPS = 1e-6
SCALE = float(C) ** -0.5
SC2 = SCALE / 65536.0
LN128 = float(np.log(128.0))

_NC_CACHE = {}
LAST_RESULT = None


def _build_nc():
    import concourse.bacc as bacc
    import concourse.tile as tile
    from concourse import mybir
    from concourse.bass import ts

    F32 = mybir.dt.float32
    BF16 = mybir.dt.bfloat16
    AX = mybir.AxisListType
    AF = mybir.ActivationFunctionType
    OP = mybir.AluOpType

    nc = bacc.Bacc(None, target_bir_lowering=False, num_swdge_queues=4)

    xsb_d = nc.dram_tensor("xsb", [BB, C, N], BF16, kind="ExternalInput")
    wqt16_d = nc.dram_tensor("wqt16", [C, C], BF16, kind="ExternalInput")
    wkt16_d = nc.dram_tensor("wkt16", [C, C], BF16, kind="ExternalInput")
    wvt_d = nc.dram_tensor("wvtb", [C, C], BF16, kind="ExternalInput")
    wvr_d = nc.dram_tensor("wvrb", [C, C], BF16, kind="ExternalInput")
    wot_d = nc.dram_tensor("wotb", [C, C], BF16, kind="ExternalInput")
    bq256_d = nc.dram_tensor("bq256", [C], F32, kind="ExternalInput")
    bk256_d = nc.dram_tensor("bk256", [C], F32, kind="ExternalInput")
    bv_d = nc.dram_tensor("bv", [C], F32, kind="ExternalInput")
    bo_d = nc.dram_tensor("bo", [C], F32, kind="ExternalInput")
    gamma_d = nc.dram_tensor("gamma", [C], F32, kind="ExternalInput")
    beta_d = nc.dram_tensor("beta", [C], F32, kind="ExternalInput")
    gfwd_d = nc.dram_tensor("gfwd", [P, CB, GROUPS], BF16, kind="ExternalInput")
    gbwd_d = nc.dram_tensor("gbwd", [GROUPS, CB, P], BF16, kind="ExternalInput")
    identbf_d = nc.dram_tensor("identbf", [P, P], BF16, kind="ExternalInput")
    y_d = nc.dram_tensor("y", [BB, C, N], BF16, kind="ExternalOutput")

    with tile.TileContext(nc) as tc:
        with (
            tc.tile_pool(name="singles", bufs=1) as sg,
            tc.tile_pool(name="sbp", bufs=1) as sbp,
            tc.tile_pool(name="psp", bufs=1, space="PSUM") as psp,
            tc.tile_pool(name="drp", bufs=1, space="DRAM") as drp,
        ):
            xbview = [xsb_d[b].rearrange("(cb p) n -> p cb n", p=P) for b in range(BB)]
            yview = [y_d[b].rearrange("(ob p) n -> p ob n", p=P) for b in range(BB)]
            st = [dict() for _ in range(BB)]  # per-batch tile state

            def emit_load(b, chunks):
                s = st[b]
                xbf = sbp.tile([P, CB, N], BF16, tag="xbf", bufs=2, name=f"xbf{b}")
                s["xbf"] = xbf
                # chunk-major so early pixel tiles land first
                for ch in range(chunks):
                    for cb in range(CB):
                        nc.sync.dma_start(
                            xbf[:, cb, ts(ch, N // chunks)],
                            xbview[b][:, cb, ts(ch, N // chunks)],
                        )

            def emit_gram(b):
                """xT (pixel-major via PE transpose matmul, plus a ones
                column) -> raw Gram with per-channel rowsums riding along.
                Also kicks the block-0 rowsum reduce on gpsimd."""
                s = st[b]
                xbf = s["xbf"]
                s8 = sbp.tile([P, 8], F32, tag="s8", bufs=2, name=f"s8{b}")
                s1_0 = sbp.tile([P, 1], F32, tag="s1_0", bufs=2, name=f"s10{b}")
                for j in range(8):
                    nc.vector.reduce_sum(
                        s8[:, j : j + 1], xbf[:, 0, ts(j, 512)], AX.X
                    )
                nc.vector.reduce_sum(s1_0, s8, AX.X)
                s["s1_0"] = s1_0
                pG = [
                    psp.tile(
                        [P, C - a * P + (1 if a else 0)], F32,
                        tag="scores", bufs=4, name=f"pG{b}_{a}",
                    )
                    for a in range(CB)
                ]
                s["pG"] = pG
                for half in range(NT // NTH):
                    for ih in range(NTH):
                        i = half * NTH + ih
                        pT = psp.tile([P, C], F32, tag="work", bufs=4,
                                      name=f"pT{b}_{i}")
                        for cb in range(CB):
                            nc.tensor.matmul(
                                pT[:, ts(cb, P)], xbf[:, cb, ts(i, P)],
                                identbf, start=True, stop=True,
                            )
                        if ih % 2 == 0:
                            nc.scalar.copy(hnT[:, ih, :C], pT)
                        else:
                            nc.vector.tensor_copy(hnT[:, ih, :C], pT)
                    for ih in range(NTH):
                        i = half * NTH + ih
                        for a in range(CB):
                            nc.tensor.matmul(
                                pG[a], hnT[:, ih, ts(a, P)],
                                hnT[:, ih, a * P : C + (1 if a else 0)],
                                start=(i == 0), stop=(i == NT - 1),
                            )
                Gb = sbp.tile([P, CB, C], BF16, tag="Gb", bufs=1, name=f"Gb{b}")
                for a in range(CB):
                    nc.scalar.copy(Gb[:, a, a * P :], pG[a][:, : C - a * P])
                # mirror the 6 sub-diagonal blocks: G[b,a] = G[a,b]^T
                for a in range(CB):
                    for bb2 in range(a + 1, CB):
                        nc.sync.dma_start(
                            Gb[:, bb2, ts(a, P)],
                            Gb[:, a, ts(bb2, P)],
                            transpose=True,
                        )
                s["Gb"] = Gb

            def emit_stats(b):
                """Per-channel [sum(x), sum(x^2)] -> t_hi/t_lo (bf16 split)
                from the Gram diagonal + ones-column rowsums."""
                s = st[b]
                pG, s1_0 = s["pG"], s["s1_0"]
                t32 = sbp.tile([P, CB, 2], F32, tag="t32", bufs=2, name=f"t{b}")
                nc.vector.tensor_copy(t32[:, 0, 0:1], s1_0)
                for a in range(1, CB):
                    w = C - a * P
                    nc.vector.tensor_copy(t32[:, a, 0:1], pG[a][:, w : w + 1])
                for a in range(CB):
                    nc.vector.tensor_tensor(
                        ttrj, pG[a][:, :P], identbf, op=OP.mult
                    )
                    nc.vector.reduce_sum(t32[:, a, 1:2], ttrj, AX.X)
                t_hi = sbp.tile([P, CB, 2], BF16, tag="t_hi", bufs=2,
                                name=f"th{b}")
                t_lo = sbp.tile([P, CB, 2], BF16, tag="t_lo", bufs=2,
                                name=f"tl{b}")
                nc.vector.tensor_copy(t_hi, t32)
                nc.vector.tensor_tensor(t_lo, t32, t_hi, op=OP.subtract)
                s["t32"], s["t_hi"], s["t_lo"] = t32, t_hi, t_lo

            def emit_a2(b):
                """Group aggregation -> A, B; A-scaled wq/wk; biases;
                scores rank-1 correction vectors. bf16 GEMMs with hi/lo
                operand splitting."""
                s = st[b]
                t32, t_hi, t_lo = s["t32"], s["t_hi"], s["t_lo"]
                pg = psp.tile([GROUPS, 2], F32, tag="work", bufs=4, name=f"pg{b}")
                for cb in range(CB):
                    nc.tensor.matmul(
                        pg, gfwd[:, cb, :], t_hi[:, cb, :],
                        start=(cb == 0), stop=False,
                    )
                    nc.tensor.matmul(
                        pg, gfwd[:, cb, :], t_lo[:, cb, :],
                        start=False, stop=(cb == CB - 1),
                    )
                gs = sbp.tile([GROUPS, 2], F32, tag="gs", bufs=2, name=f"gs{b}")
                pgs = sbp.tile([GROUPS, 2], F32, tag="pgs", bufs=2, name=f"pgs{b}")
                nc.vector.tensor_copy(pgs, pg)
                vtmp = sbp.tile([GROUPS, 1], F32, tag="vtmp", bufs=2, name=f"vt{b}")
                nc.vector.tensor_mul(vtmp, pgs[:, 0:1], pgs[:, 0:1])
                nc.vector.tensor_tensor(vtmp, pgs[:, 1:2], vtmp, op=OP.subtract)
                nc.vector.tensor_copy(gs[:, 0:1], pgs[:, 0:1])
                nc.scalar.activation(gs[:, 1:2], vtmp, AF.Sqrt, bias=eps_g)
                nc.vector.reciprocal(gs[:, 1:2], gs[:, 1:2])
                gs_hi = sbp.tile([GROUPS, 2], BF16, tag="gs_hi", bufs=2,
                                 name=f"gh{b}")
                gs_lo = sbp.tile([GROUPS, 2], BF16, tag="gs_lo", bufs=2,
                                 name=f"gl{b}")
                nc.vector.tensor_copy(gs_hi, gs)
                nc.vector.tensor_tensor(gs_lo, gs, gs_hi, op=OP.subtract)

                cst = sbp.tile([P, CB, 2], F32, tag="cst", bufs=2, name=f"cs{b}")
                for cb in range(CB):
                    pc = psp.tile([P, 2], F32, tag="work", bufs=4, name=f"pc{b}_{cb}")
                    nc.tensor.matmul(pc, gbwd[:, cb, :], gs_hi,
                                     start=True, stop=False)
                    nc.tensor.matmul(pc, gbwd[:, cb, :], gs_lo,
                                     start=False, stop=True)
                    nc.vector.tensor_copy(cst[:, cb, :], pc)

                A_ = sbp.tile([P, CB], F32, tag="A_", bufs=2, name=f"A{b}")
                # brs packs (B, rs) as two lhsT columns for the matvec GEMMs
                brs = sbp.tile([P, CB, 2], BF16, tag="brs", bufs=2, name=f"brs{b}")
                tmpB = sbp.tile([P, CB], F32, tag="tmpB", bufs=2, name=f"tB{b}")
                nc.vector.tensor_mul(A_, cst[:, :, 1], gam)
                nc.vector.tensor_mul(tmpB, cst[:, :, 0], A_)
                nc.vector.tensor_tensor(brs[:, :, 0], bet, tmpB, op=OP.subtract)
                s["A_"] = A_

                # per-batch A-scaled weights for the Gram->scores GEMMs
                A16 = sbp.tile([P, CB], F32, tag="A16", bufs=2, name=f"A16{b}")
                nc.vector.tensor_scalar_mul(A16, A_, 16.0)
                wqa = sbp.tile([P, CB, C], BF16, tag="wqa", bufs=2, name=f"wqa{b}")
                wka = sbp.tile([P, CB, C], BF16, tag="wka", bufs=2, name=f"wka{b}")
                for cb in range(CB):
                    nc.vector.tensor_scalar_mul(
                        wqa[:, cb, :], wqt16[:, cb, :], A16[:, cb : cb + 1]
                    )
                    nc.vector.tensor_scalar_mul(
                        wka[:, cb, :], wkt16[:, cb, :], A16[:, cb : cb + 1]
                    )
                s["wqa"], s["wka"] = wqa, wka

                # v bias: bvb = bv + Wv@B, via DRAM round-trip to [P, CB]
                pb = psp.tile([1, C], F32, tag="work", bufs=4, name=f"pbv{b}")
                for cb in range(CB):
                    nc.tensor.matmul(
                        pb, brs[:, cb, 0:1], wvt[:, cb, :],
                        start=(cb == 0), stop=(cb == CB - 1),
                    )
                bvrow = sbp.tile([1, C], F32, tag="bvrow", bufs=2, name=f"bvr{b}")
                nc.vector.tensor_add(bvrow, pb, bvv)
                scr = drp.tile([C], F32, name=f"scrv{b}")
                nc.sync.dma_start(scr.rearrange("(a c) -> a c", a=1), bvrow)
                bvb = sbp.tile([P, CB], F32, tag="bvb", bufs=2, name=f"bvb{b}")
                nc.sync.dma_start(bvb, scr.rearrange("(cb p) -> p cb", p=P))
                bvb16 = sbp.tile([P, CB], BF16, tag="bvb16", bufs=2,
                                 name=f"bvb16{b}")
                nc.vector.tensor_copy(bvb16, bvb)
                s["bvb"] = bvb16

                # scores rank-1 vectors (x256 scale):
                #   cq256 = 256*(Wq@B + bq), sq256 = 256*(Wq@rs),
                #   rs = A*sum(x) (raw sums, so 16.0 not 16N)
                rsf = sbp.tile([P, CB], F32, tag="rsf", bufs=2, name=f"rsf{b}")
                nc.vector.tensor_mul(rsf, A_, t32[:, :, 0])
                nc.vector.tensor_scalar_mul(brs[:, :, 1], rsf, 16.0)
                rows = {}
                for nm, wt, brow in (("q", wqt16, bq256r), ("k", wkt16, bk256r)):
                    pc1 = psp.tile([1, C], F32, tag="work", bufs=4,
                                   name=f"pc1{b}{nm}")
                    for cb in range(CB):
                        nc.tensor.matmul(
                            pc1, brs[:, cb, 0:1], wt[:, cb, :],
                            start=(cb == 0), stop=(cb == CB - 1),
                        )
                    crow = sbp.tile([1, C], BF16, tag=f"c{nm}row", bufs=2,
                                    name=f"c{nm}{b}")
                    tmpr = sbp.tile([1, C], F32, tag="tmpr", bufs=2,
                                    name=f"tr{b}{nm}")
                    nc.vector.tensor_scalar_mul(tmpr, pc1, 16.0)
                    nc.vector.tensor_add(crow, tmpr, brow)
                    rows[f"c{nm}"] = crow
                    ps1 = psp.tile([1, C], F32, tag="work", bufs=4,
                                   name=f"ps1{b}{nm}")
                    for cb in range(CB):
                        nc.tensor.matmul(
                            ps1, brs[:, cb, 1:2], wt[:, cb, :],
                            start=(cb == 0), stop=(cb == CB - 1),
                        )
                    srow = sbp.tile([1, C], BF16, tag=f"s{nm}row", bufs=2,
                                    name=f"s{nm}{b}")
                    nc.vector.tensor_copy(srow, ps1)
                    rows[f"s{nm}"] = srow
                rhs1 = sbp.tile([1, C], BF16, tag="rhs1", bufs=2, name=f"rh{b}")
                nc.vector.tensor_scalar_mul(rhs1, rows["ck"], float(N))
                nc.vector.tensor_add(rhs1, rhs1, rows["sk"])
                s["cq"], s["sq"], s["ck"] = rows["cq"], rows["sq"], rows["ck"]
                s["rhs1"] = rhs1

            def emit_t1t(b):
                """T1T[d, o] = sum_c G[d,c] wqa[o,c] — G is symmetric, so
                Gb blocks serve as lhsT directly; no transpose pass."""
                s = st[b]
                Gb, wqa = s["Gb"], s["wqa"]
                T1T = sbp.tile([P, CB, C], BF16, tag="T1b", bufs=1, name=f"TT{b}")
                s["T1T"] = T1T
                for dcb in range(CB):
                    pT1 = psp.tile([P, C], F32, tag="work", bufs=4,
                                   name=f"pT1{b}_{dcb}")
                    for cb in range(CB):
                        nc.tensor.matmul(
                            pT1, Gb[:, cb, ts(dcb, P)], wqa[:, cb, :],
                            start=(cb == 0), stop=(cb == CB - 1),
                        )
                    nc.scalar.copy(T1T[:, dcb, :], pT1)

            def emit_scores(b):
                """scores[o, e] = sum_d T1T[d, o] wka[d, e] + rank-1."""
                s = st[b]
                T1T, wka = s["T1T"], s["wka"]
                cq, sq, ck, rhs1 = s["cq"], s["sq"], s["ck"], s["rhs1"]
                scores = [
                    psp.tile([P, C], F32, tag="scores", bufs=4, name=f"sc{b}_{cb}")
                    for cb in range(CB)
                ]
                s["scores"] = scores
                for ocb in range(CB):
                    for db in range(CB):
                        nc.tensor.matmul(
                            scores[ocb], T1T[:, db, ts(ocb, P)], wka[:, db, :],
                            start=(db == 0), stop=False,
                        )
                    nc.tensor.matmul(
                        scores[ocb], cq[:, ts(ocb, P)], rhs1,
                        start=False, stop=False,
                    )
                    nc.tensor.matmul(
                        scores[ocb], sq[:, ts(ocb, P)], ck,
                        start=False, stop=True,
                    )

            def emit_softmax(b):
                """Max-subtracted exp (x128), row sums -> rinv."""
                s = st[b]
                scores = s["scores"]
                e_sb = sbp.tile([P, CB, C], BF16, tag="e", bufs=1, name=f"e{b}")
                rinv = sbp.tile([P, CB], F32, tag="rinv", bufs=1, name=f"ri{b}")
                rmx = sbp.tile([P, CB], F32, tag="rmx", bufs=1, name=f"rm{b}")
                eb = sbp.tile([P, CB], F32, tag="eb", bufs=1, name=f"eb{b}")
                rsum = sbp.tile([P, CB], F32, tag="rsum", bufs=1, name=f"rs{b}")
                s["e"], s["rinv"] = e_sb, rinv
                for cb in range(CB):
                    nc.vector.reduce_max(
                        rmx[:, cb : cb + 1], scores[cb], axis=AX.X
                    )
                    nc.vector.tensor_scalar(
                        eb[:, cb : cb + 1], rmx[:, cb : cb + 1],
                        -SC2, LN128, op0=OP.mult, op1=OP.add,
                    )
                    nc.scalar.activation(
                        e_sb[:, cb, :], scores[cb], AF.Exp,
                        bias=eb[:, cb : cb + 1], scale=SC2,
                        accum_out=rsum[:, cb : cb + 1],
                    )
                    nc.vector.reciprocal(
                        rinv[:, cb : cb + 1], rsum[:, cb : cb + 1]
                    )

            def emit_m(b):
                """R = (Wo attn)^T = e^T (rinv*Wo^T); r = R^T bvb + bo;
                M'^T[e,o] = I + A[e] * sum_d Wv[d,e] R[d,o]."""
                s = st[b]
                e_sb, rinv, bvb, A_ = s["e"], s["rinv"], s["bvb"], s["A_"]
                wotr = sbp.tile([P, CB, C], BF16, tag="wotr", bufs=2,
                                name=f"wr{b}")
                for cb in range(CB):
                    nc.vector.tensor_scalar_mul(
                        wotr[:, cb, :], wot[:, cb, :], rinv[:, cb : cb + 1]
                    )
                Rb = sbp.tile([P, CB, C], BF16, tag="Rb", bufs=2, name=f"Rb{b}")
                for db in range(CB):
                    pR = psp.tile([P, C], F32, tag="work", bufs=4,
                                  name=f"pR{b}{db}")
                    for cb in range(CB):
                        nc.tensor.matmul(
                            pR, e_sb[:, cb, ts(db, P)], wotr[:, cb, :],
                            start=(cb == 0), stop=(cb == CB - 1),
                        )
                    nc.scalar.copy(Rb[:, db, :], pR)
                # r[o] = sum_d R[d, o] bvb[d] + bo, per-partition layout
                pr = psp.tile([P, CB], F32, tag="work", bufs=4, name=f"pr{b}")
                for ob in range(CB):
                    for db in range(CB):
                        nc.tensor.matmul(
                            pr[:, ob : ob + 1], Rb[:, db, ts(ob, P)],
                            bvb[:, db : db + 1],
                            start=(db == 0), stop=(db == CB - 1),
                        )
                rb = sbp.tile([P, CB], F32, tag="rb", bufs=2, name=f"rv{b}")
                nc.vector.tensor_add(rb, pr, bob)
                s["rb"] = rb
                MtT = sbp.tile([P, CB, C], BF16, tag="MtT", bufs=2,
                               name=f"Mt{b}")
                for eb2 in range(CB):
                    pM = psp.tile([P, C], F32, tag="work", bufs=4,
                                  name=f"pM{b}{eb2}")
                    for db in range(CB):
                        nc.tensor.matmul(
                            pM, wvr[:, db, ts(eb2, P)], Rb[:, db, :],
                            start=(db == 0), stop=(db == CB - 1),
                        )
                    nc.scalar.mul(MtT[:, eb2, :], pM, A_[:, eb2 : eb2 + 1])
                # fold the residual: M' = M + I (diagonal blocks)
                for eb2 in range(CB):
                    nc.vector.tensor_add(
                        MtT[:, eb2, ts(eb2, P)], MtT[:, eb2, ts(eb2, P)],
                        identbf,
                    )
                s["MtT"] = MtT

            def emit_y(b, nsls):
                """Y = M' x + r 1^T for the given pixel slices (bf16 out,
                paired output DMAs alternating sync/gpsimd queues)."""
                s = st[b]
                xbf, MtT, rb = s["xbf"], s["MtT"], s["rb"]
                for nsl in nsls:
                    for ob in range(CB):
                        pf = psp.tile([P, NSL], F32, tag="work", bufs=4,
                                      name=f"pf{b}{nsl}{ob}")
                        for eb2 in range(CB):
                            nc.tensor.matmul(
                                pf, MtT[:, eb2, ts(ob, P)],
                                xbf[:, eb2, ts(nsl, NSL)],
                                start=(eb2 == 0), stop=(eb2 == CB - 1),
                            )
                        yt = sbp.tile([P, NSL], BF16, tag="yt", bufs=4,
                                      name=f"yt{b}{nsl}{ob}")
                        if (nsl * CB + ob) % 2 == 0:
                            nc.vector.tensor_scalar_add(
                                yt, pf, rb[:, ob : ob + 1]
                            )
                        else:
                            nc.scalar.add(yt, pf, rb[:, ob : ob + 1])
                        nc.sync.dma_start(yview[b][:, ob, ts(nsl, NSL)], yt)

            # ---- identbf first (unblocks gram(0)), then x chunks ----
            identbf = sg.tile([P, P], BF16)
            nc.sync.dma_start(identbf, identbf_d[:])
            emit_load(0, NCH)
            # small consts + weights on the gpsimd queue
            gfwd = sg.tile([P, CB, GROUPS], BF16)
            nc.sync.dma_start(gfwd, gfwd_d[:])
            gbwd = sg.tile([GROUPS, CB, P], BF16)
            nc.sync.dma_start(gbwd, gbwd_d[:])
            gam = sg.tile([P, CB], F32)
            nc.sync.dma_start(gam, gamma_d[:].rearrange("(cb p) -> p cb", p=P))
            bet = sg.tile([P, CB], F32)
            nc.sync.dma_start(bet, beta_d[:].rearrange("(cb p) -> p cb", p=P))
            bob = sg.tile([P, CB], F32)
            nc.sync.dma_start(bob, bo_d[:].rearrange("(cb p) -> p cb", p=P))
            bq256r = sg.tile([1, C], F32)
            nc.sync.dma_start(bq256r, bq256_d[:].rearrange("(a c) -> a c", a=1))
            bk256r = sg.tile([1, C], F32)
            nc.sync.dma_start(bk256r, bk256_d[:].rearrange("(a c) -> a c", a=1))
            bvv = sg.tile([1, C], F32)
            nc.sync.dma_start(bvv, bv_d[:].rearrange("(a c) -> a c", a=1))
            eps_g = sg.tile([GROUPS, 1], F32)
            nc.vector.memset(eps_g, EPS)
            # shared pixel-major tile (written per batch) + ones column
            hnT = sg.tile([P, NTH, C + 1], BF16, name="hnT")
            nc.vector.memset(hnT, 1.0)
            ttrj = sg.tile([P, P], F32, name="ttrj")
            # ---- short HAM warm-up while the first x chunks land ----
            zsb = sg.tile([P, NSL], BF16, name="zsb")
            nc.gpsimd.memset(zsb, 0.0)
            pdum = psp.tile([P, NSL], F32, tag="work", bufs=4, name="pdum")
            for i in range(6):
                nc.tensor.matmul(
                    pdum, zsb[:, :P], zsb, start=(i == 0), stop=False
                )
            for cb in range(CB):
                nc.tensor.matmul(
                    pdum, st[0]["xbf"][:, cb, ts(0, P)], zsb,
                    start=False, stop=(cb == CB - 1),
                )
            dsb = sg.tile([1, 1], F32, name="dsb")
            nc.vector.tensor_copy(dsb, pdum[0:1, 0:1])
            dscr = drp.tile([1], F32, name="dscr")
            nc.sync.dma_start(dscr.rearrange("(a c) -> a c", a=1), dsb)
            # ---- weight tiles (first needed by a2/t1t, ~30us in) ----
            wqt16 = sg.tile([P, CB, C], BF16)
            nc.sync.dma_start(wqt16, wqt16_d[:].rearrange("(cb p) o -> p cb o", p=P))
            wkt16 = sg.tile([P, CB, C], BF16)
            nc.sync.dma_start(wkt16, wkt16_d[:].rearrange("(cb p) o -> p cb o", p=P))
            wvt = sg.tile([P, CB, C], BF16)
            nc.sync.dma_start(wvt, wvt_d[:].rearrange("(cb p) o -> p cb o", p=P))
            wvr = sg.tile([P, CB, C], BF16)
            nc.sync.dma_start(wvr, wvr_d[:].rearrange("(cb p) o -> p cb o", p=P))
            wot = sg.tile([P, CB, C], BF16)
            nc.sync.dma_start(wot, wot_d[:].rearrange("(cb p) o -> p cb o", p=P))

            # ---- pipelined schedule (BB=2) ----
            emit_gram(0)           # starts as soon as x chunks land
            emit_stats(0)          # vector, from Gram diag + rowsums
            emit_a2(0)
            emit_load(1, 1)
            emit_t1t(0)
            emit_scores(0)
            emit_softmax(0)
            emit_gram(1)           # PE-heavy; covers softmax(0) latency
            emit_stats(1)
            emit_m(0)
            emit_a2(1)
            emit_y(0, range(0, 4))
            emit_t1t(1)            # mirror-DMA seam covered by m(0)/y(0)
            emit_scores(1)
            emit_y(0, range(4, NS))  # covers softmax(1) latency
            emit_softmax(1)
            emit_m(1)
            emit_y(1, range(NS))

    nc.finalize()
    return nc


def _get_nc():
    if "nc" not in _NC_CACHE:
        _NC_CACHE["nc"] = _build_nc()
    return _NC_CACHE["nc"]


def _make_consts():
    import ml_dtypes

    BF = ml_dtypes.bfloat16
    gfwd = np.zeros((P, CB, GROUPS), np.float32)
    gbwd = np.zeros((GROUPS, CB, P), np.float32)
    for cb in range(CB):
        for p in range(P):
            g = (cb * P + p) // 16
            gfwd[p, cb, g] = 1.0 / (16.0 * N)   # raw sums -> group means
            gbwd[g, cb, p] = 1.0
    return gfwd.astype(BF), gbwd.astype(BF)


def kernel(x, gamma, beta, Wq, bq, Wk, bk, Wv, bv, Wo, bo):
    global LAST_RESULT
    from concourse.bass_utils import run_bass_kernel_spmd

    import ml_dtypes

    BF = ml_dtypes.bfloat16
    x = np.ascontiguousarray(np.asarray(x, np.float32)).reshape(16, C, N)
    xb16 = np.ascontiguousarray(x.astype(BF))
    gfwd, gbwd = _make_consts()
    shared = {
        "wqt16": np.ascontiguousarray(
            (np.asarray(Wq, np.float32).T * 16.0).astype(BF)
        ),
        "wkt16": np.ascontiguousarray(
            (np.asarray(Wk, np.float32).T * 16.0).astype(BF)
        ),
        "wvtb": np.ascontiguousarray(np.asarray(Wv, np.float32).T.astype(BF)),
        "wvrb": np.ascontiguousarray(np.asarray(Wv, np.float32).astype(BF)),
        "wotb": np.ascontiguousarray(np.asarray(Wo, np.float32).T.astype(BF)),
        "bq256": np.ascontiguousarray(np.asarray(bq, np.float32) * 256.0),
        "bk256": np.ascontiguousarray(np.asarray(bk, np.float32) * 256.0),
        "bv": np.ascontiguousarray(np.asarray(bv, np.float32)),
        "bo": np.ascontiguousarray(np.asarray(bo, np.float32)),
        "gamma": np.ascontiguousarray(np.asarray(gamma, np.float32)),
        "beta": np.ascontiguousarray(np.asarray(beta, np.float32)),
        "gfwd": np.ascontiguousarray(gfwd),
        "gbwd": np.ascontiguousarray(gbwd),
        "identbf": np.ascontiguousarray(np.eye(P, dtype=np.float32).astype(BF)),
    }
    in_maps = [
        dict(shared, xsb=np.ascontiguousarray(xb16[BB * i : BB * (i + 1)]))
        for i in range(8)
    ]
    nc = _get_nc()
    import os

    trace = os.environ.get("KERNEL_TRACE") == "1"
    res = run_bass_kernel_spmd(nc, in_maps, core_ids=list(range(8)), trace=trace)
    LAST_RESULT = res
    y = np.concatenate(
        [np.asarray(r["y"], np.float32) for r in res.results], axis=0
    )
    return y.reshape(16, C, 64, 64)


# revision 42
# speedup vs baseline: 1.0311x; 1.0311x over previous
"""AttnBlock (channel attention over 64x64 maps) for Trainium2 — factored
epilogue + transpose-Gram + stats-from-Gram edition.

Data-parallel over batch: 16 batches, 2 per core on 8 NeuronCores.
Per batch [C=512, N=4096], hn = A*x + B (GroupNorm folded to per-channel
affine):

  scores = q^T k with q = Wq hn + bq factorizes through the RAW Gram
    matrix Graw = x @ x^T (C x C):
      scores = (256 Wq A) Graw (256 A Wk)^T / 65536 + rank-1 corrections
    with diag(A) folded into per-batch scaled weights wqa/wka, so the
    pixel-major xT build is a PURE transpose that starts as soon as the
    first x chunks land. The GroupNorm stats come from the Gram itself:
    per-channel sum(x^2) is the Gram diagonal (exact: bf16*bf16 products
    accumulate exactly in fp32) and per-channel sum(x) rides along as a
    ones-column appended to the transposed tiles (blocks 1-3; block 0's
    PSUM bank is full, its rowsum comes from a gpsimd reduce). bn_stats
    and its 22us of serial vector time are gone. The group aggregation
    runs in bf16 matmuls with hi/lo operand splitting (fp32 accuracy at
    bf16 speed).
  The epilogue factors the same way: out = Wo attn v collapses to
      y = M' x + r 1^T,  M' = I + Wo attn Wv diag(A),
      r = Wo attn bvb + bo
    via R = e^T (rinv*Wo^T), M^T = Wv R (A-scaled at eviction, identity
    added), then ONE C*C*N application off the resident bf16 x, evicted
    bf16 with vector/scalar alternation and paired output DMAs spread
    over the sync and gpsimd queues.
Batches are software-pipelined; batch1's Gram covers batch0's softmax;
batch0's Y GEMM is split around batch1's t1t/scores to cover seams.
"""

import sys

if "/opt/trn_rl_repo" not in sys.path:
    sys.path.insert(0, "/opt/trn_rl_repo")

import numpy as np

C = 512          # channels
N = 4096         # pixels (64*64)
BB = 2           # batches per core
P = 128          # partitions
CB = C // P      # 4 channel blocks
NT = N // P      # 32 pixel tiles of 128
NTH = 8          # pixel tiles per hnT chunk
NSL = 512        # pixel slice width (y phase)
NS = N // NSL    # 8 pixel slices
NCH = 4          # x load chunks (batch 0)
GROUPS = 32
EPS = 1e-6
SCALE = float(C) ** -0.5
SC2 = SCALE / 65536.0
LN128 = float(np.log(128.0))

_NC_CACHE = {}
LAST_RESULT = None


def _build_nc():
    import concourse.bacc as bacc
    import concourse.tile as tile
    from concourse import mybir
    from concourse.bass import ts

    F32 = mybir.dt.float32
    BF16 = mybir.dt.bfloat16
    AX = mybir.AxisListType
    AF = mybir.ActivationFunctionType
    OP = mybir.AluOpType

    nc = bacc.Bacc(None, target_bir_lowering=False, num_swdge_queues=4)

    xsb_d = nc.dram_tensor("xsb", [BB, C, N], BF16, kind="ExternalInput")
    wqt16_d = nc.dram_tensor("wqt16", [C, C], BF16, kind="ExternalInput")
    wkt16_d = nc.dram_tensor("wkt16", [C, C], BF16, kind="ExternalInput")
    wvt_d = nc.dram_tensor("wvtb", [C, C], BF16, kind="ExternalInput")
    wvr_d = nc.dram_tensor("wvrb", [C, C], BF16, kind="ExternalInput")
    wot_d = nc.dram_tensor("wotb", [C, C], BF16, kind="ExternalInput")
    bq256_d = nc.dram_tensor("bq256", [C], F32, kind="ExternalInput")
    bk256_d = nc.dram_tensor("bk256", [C], F32, kind="ExternalInput")
    bv_d = nc.dram_tensor("bv", [C], F32, kind="ExternalInput")
    bo_d = nc.dram_tensor("bo", [C], F32, kind="ExternalInput")
    gamma_d = nc.dram_tensor("gamma", [C], F32, kind="ExternalInput")
    beta_d = nc.dram_tensor("beta", [C], F32, kind="ExternalInput")
    gfwd_d = nc.dram_tensor("gfwd", [P, CB, GROUPS], BF16, kind="ExternalInput")
    gbwd_d = nc.dram_tensor("gbwd", [GROUPS, CB, P], BF16, kind="ExternalInput")
    identbf_d = nc.dram_tensor("identbf", [P, P], BF16, kind="ExternalInput")
    y_d = nc.dram_tensor("y", [BB, C, N], BF16, kind="ExternalOutput")

    with tile.TileContext(nc) as tc:
        with (
            tc.tile_pool(name="singles", bufs=1) as sg,
            tc.tile_pool(name="sbp", bufs=1) as sbp,
            tc.tile_pool(name="psp", bufs=1, space="PSUM") as psp,
            tc.tile_pool(name="drp", bufs=1, space="DRAM") as drp,
        ):
            xbview = [xsb_d[b].rearrange("(cb p) n -> p cb n", p=P) for b in range(BB)]
            yview = [y_d[b].rearrange("(ob p) n -> p ob n", p=P) for b in range(BB)]
            st = [dict() for _ in range(BB)]  # per-batch tile state

            def emit_load(b, chunks):
                s = st[b]
                xbf = sbp.tile([P, CB, N], BF16, tag="xbf", bufs=2, name=f"xbf{b}")
                s["xbf"] = xbf
                # chunk-major so early pixel tiles land first
                for ch in range(chunks):
                    for cb in range(CB):
                        nc.sync.dma_start(
                            xbf[:, cb, ts(ch, N // chunks)],
                            xbview[b][:, cb, ts(ch, N // chunks)],
                        )

            def emit_gram(b):
                """xT (pixel-major via PE transpose matmul, plus a ones
                column) -> raw Gram with per-channel rowsums riding along.
                Also kicks the block-0 rowsum reduce on gpsimd."""
                s = st[b]
                xbf = s["xbf"]
                s8 = sbp.tile([P, 8], F32, tag="s8", bufs=2, name=f"s8{b}")
                s1_0 = sbp.tile([P, 1], F32, tag="s1_0", bufs=2, name=f"s10{b}")
                for j in range(8):
                    nc.vector.reduce_sum(
                        s8[:, j : j + 1], xbf[:, 0, ts(j, 512)], AX.X
                    )
                nc.vector.reduce_sum(s1_0, s8, AX.X)
                s["s1_0"] = s1_0
                pG = [
                    psp.tile(
                        [P, C - a * P + (1 if a else 0)], F32,
                        tag="scores", bufs=4, name=f"pG{b}_{a}",
                    )
                    for a in range(CB)
                ]
                s["pG"] = pG
                for half in range(NT // NTH):
                    for ih in range(NTH):
                        i = half * NTH + ih
                        pT = psp.tile([P, C], F32, tag="work", bufs=4,
                                      name=f"pT{b}_{i}")
                        for cb in range(CB):
                            nc.tensor.matmul(
                                pT[:, ts(cb, P)], xbf[:, cb, ts(i, P)],
                                identbf, start=True, stop=True,
                            )
                        if ih % 2 == 0:
                            nc.scalar.copy(hnT[:, ih, :C], pT)
                        else:
                            nc.vector.tensor_copy(hnT[:, ih, :C], pT)
                    for ih in range(NTH):
                        i = half * NTH + ih
                        for a in range(CB):
                            nc.tensor.matmul(
                                pG[a], hnT[:, ih, ts(a, P)],
                                hnT[:, ih, a * P : C + (1 if a else 0)],
                                start=(i == 0), stop=(i == NT - 1),
                            )
                Gb = sbp.tile([P, CB, C], BF16, tag="Gb", bufs=1, name=f"Gb{b}")
                for a in range(CB):
                    nc.scalar.copy(Gb[:, a, a * P :], pG[a][:, : C - a * P])
                # mirror the 6 sub-diagonal blocks: G[b,a] = G[a,b]^T
                for a in range(CB):
                    for bb2 in range(a + 1, CB):
                        nc.sync.dma_start(
                            Gb[:, bb2, ts(a, P)],
                            Gb[:, a, ts(bb2, P)],
                            transpose=True,
                        )
                s["Gb"] = Gb

            def emit_stats(b):
                """Per-channel [sum(x), sum(x^2)] -> t_hi/t_lo (bf16 split)
                from the Gram diagonal + ones-column rowsums."""
                s = st[b]
                pG, s1_0 = s["pG"], s["s1_0"]
                t32 = sbp.tile([P, CB, 2], F32, tag="t32", bufs=2, name=f"t{b}")
                nc.vector.tensor_copy(t32[:, 0, 0:1], s1_0)
                for a in range(1, CB):
                    w = C - a * P
                    nc.vector.tensor_copy(t32[:, a, 0:1], pG[a][:, w : w + 1])
                for a in range(CB):
                    nc.vector.tensor_tensor(
                        ttrj, pG[a][:, :P], identbf, op=OP.mult
                    )
                    nc.vector.reduce_sum(t32[:, a, 1:2], ttrj, AX.X)
                t_hi = sbp.tile([P, CB, 2], BF16, tag="t_hi", bufs=2,
                                name=f"th{b}")
                t_lo = sbp.tile([P, CB, 2], BF16, tag="t_lo", bufs=2,
                                name=f"tl{b}")
                nc.vector.tensor_copy(t_hi, t32)
                nc.vector.tensor_tensor(t_lo, t32, t_hi, op=OP.subtract)
                s["t32"], s["t_hi"], s["t_lo"] = t32, t_hi, t_lo

            def emit_a2(b):
                """Group aggregation -> A, B; A-scaled wq/wk; biases;
                scores rank-1 correction vectors. bf16 GEMMs with hi/lo
                operand splitting."""
                s = st[b]
                t32, t_hi, t_lo = s["t32"], s["t_hi"], s["t_lo"]
                pg = psp.tile([GROUPS, 2], F32, tag="work", bufs=4, name=f"pg{b}")
                for cb in range(CB):
                    nc.tensor.matmul(
                        pg, gfwd[:, cb, :], t_hi[:, cb, :],
                        start=(cb == 0), stop=False,
                    )
                    nc.tensor.matmul(
                        pg, gfwd[:, cb, :], t_lo[:, cb, :],
                        start=False, stop=(cb == CB - 1),
                    )
                gs = sbp.tile([GROUPS, 2], F32, tag="gs", bufs=2, name=f"gs{b}")
                pgs = sbp.tile([GROUPS, 2], F32, tag="pgs", bufs=2, name=f"pgs{b}")
                nc.vector.tensor_copy(pgs, pg)
                vtmp = sbp.tile([GROUPS, 1], F32, tag="vtmp", bufs=2, name=f"vt{b}")
                nc.vector.tensor_mul(vtmp, pgs[:, 0:1], pgs[:, 0:1])
                nc.vector.tensor_tensor(vtmp, pgs[:, 1:2], vtmp, op=OP.subtract)
                nc.vector.tensor_copy(gs[:, 0:1], pgs[:, 0:1])
                nc.scalar.activation(gs[:, 1:2], vtmp, AF.Sqrt, bias=eps_g)
                nc.vector.reciprocal(gs[:, 1:2], gs[:, 1:2])
                gs_hi = sbp.tile([GROUPS, 2], BF16, tag="gs_hi", bufs=2,
                                 name=f"gh{b}")
                gs_lo = sbp.tile([GROUPS, 2], BF16, tag="gs_lo", bufs=2,
                                 name=f"gl{b}")
                nc.vector.tensor_copy(gs_hi, gs)
                nc.vector.tensor_tensor(gs_lo, gs, gs_hi, op=OP.subtract)

                cst = sbp.tile([P, CB, 2], F32, tag="cst", bufs=2, name=f"cs{b}")
                for cb in range(CB):
                    pc = psp.tile([P, 2], F32, tag="work", bufs=4, name=f"pc{b}_{cb}")
                    nc.tensor.matmul(pc, gbwd[:, cb, :], gs_hi,
                                     start=True, stop=False)
                    nc.tensor.matmul(pc, gbwd[:, cb, :], gs_lo,
                                     start=False, stop=True)
                    nc.vector.tensor_copy(cst[:, cb, :], pc)

                A_ = sbp.tile([P, CB], F32, tag="A_", bufs=2, name=f"A{b}")
                # brs packs (B, rs) as two lhsT columns for the matvec GEMMs
                brs = sbp.tile([P, CB, 2], BF16, tag="brs", bufs=2, name=f"brs{b}")
                tmpB = sbp.tile([P, CB], F32, tag="tmpB", bufs=2, name=f"tB{b}")
                nc.vector.tensor_mul(A_, cst[:, :, 1], gam)
                nc.vector.tensor_mul(tmpB, cst[:, :, 0], A_)
                nc.vector.tensor_tensor(brs[:, :, 0], bet, tmpB, op=OP.subtract)
                s["A_"] = A_

                # per-batch A-scaled weights for the Gram->scores GEMMs
                A16 = sbp.tile([P, CB], F32, tag="A16", bufs=2, name=f"A16{b}")
                nc.vector.tensor_scalar_mul(A16, A_, 16.0)
                wqa = sbp.tile([P, CB, C], BF16, tag="wqa", bufs=2, name=f"wqa{b}")
                wka = sbp.tile([P, CB, C], BF16, tag="wka", bufs=2, name=f"wka{b}")
                for cb in range(CB):
                    nc.vector.tensor_scalar_mul(
                        wqa[:, cb, :], wqt16[:, cb, :], A16[:, cb : cb + 1]
                    )
                    nc.vector.tensor_scalar_mul(
                        wka[:, cb, :], wkt16[:, cb, :], A16[:, cb : cb + 1]
                    )
                s["wqa"], s["wka"] = wqa, wka

                # v bias: bvb = bv + Wv@B, via DRAM round-trip to [P, CB]
                pb = psp.tile([1, C], F32, tag="work", bufs=4, name=f"pbv{b}")
                for cb in range(CB):
                    nc.tensor.matmul(
                        pb, brs[:, cb, 0:1], wvt[:, cb, :],
                        start=(cb == 0), stop=(cb == CB - 1),
                    )
                bvrow = sbp.tile([1, C], F32, tag="bvrow", bufs=2, name=f"bvr{b}")
                nc.vector.tensor_add(bvrow, pb, bvv)
                scr = drp.tile([C], F32, name=f"scrv{b}")
                nc.sync.dma_start(scr.rearrange("(a c) -> a c", a=1), bvrow)
                bvb = sbp.tile([P, CB], F32, tag="bvb", bufs=2, name=f"bvb{b}")
                nc.sync.dma_start(bvb, scr.rearrange("(cb p) -> p cb", p=P))
                bvb16 = sbp.tile([P, CB], BF16, tag="bvb16", bufs=2,
                                 name=f"bvb16{b}")
                nc.vector.tensor_copy(bvb16, bvb)
                s["bvb"] = bvb16

                # scores rank-1 vectors (x256 scale):
                #   cq256 = 256*(Wq@B + bq), sq256 = 256*(Wq@rs),
                #   rs = A*sum(x) (raw sums, so 16.0 not 16N)
                rsf = sbp.tile([P, CB], F32, tag="rsf", bufs=2, name=f"rsf{b}")
                nc.vector.tensor_mul(rsf, A_, t32[:, :, 0])
                nc.vector.tensor_scalar_mul(brs[:, :, 1], rsf, 16.0)
                rows = {}
                for nm, wt, brow in (("q", wqt16, bq256r), ("k", wkt16, bk256r)):
                    pc1 = psp.tile([1, C], F32, tag="work", bufs=4,
                                   name=f"pc1{b}{nm}")
                    for cb in range(CB):
                        nc.tensor.matmul(
                            pc1, brs[:, cb, 0:1], wt[:, cb, :],
                            start=(cb == 0), stop=(cb == CB - 1),
                        )
                    crow = sbp.tile([1, C], BF16, tag=f"c{nm}row", bufs=2,
                                    name=f"c{nm}{b}")
                    tmpr = sbp.tile([1, C], F32, tag="tmpr", bufs=2,
                                    name=f"tr{b}{nm}")
                    nc.vector.tensor_scalar_mul(tmpr, pc1, 16.0)
                    nc.vector.tensor_add(crow, tmpr, brow)
                    rows[f"c{nm}"] = crow
                    ps1 = psp.tile([1, C], F32, tag="work", bufs=4,
                                   name=f"ps1{b}{nm}")
                    for cb in range(CB):
                        nc.tensor.matmul(
                            ps1, brs[:, cb, 1:2], wt[:, cb, :],
                            start=(cb == 0), stop=(cb == CB - 1),
                        )
                    srow = sbp.tile([1, C], BF16, tag=f"s{nm}row", bufs=2,
                                    name=f"s{nm}{b}")
                    nc.vector.tensor_copy(srow, ps1)
                    rows[f"s{nm}"] = srow
                rhs1 = sbp.tile([1, C], BF16, tag="rhs1", bufs=2, name=f"rh{b}")
                nc.vector.tensor_scalar_mul(rhs1, rows["ck"], float(N))
                nc.vector.tensor_add(rhs1, rhs1, rows["sk"])
                s["cq"], s["sq"], s["ck"] = rows["cq"], rows["sq"], rows["ck"]
                s["rhs1"] = rhs1

            def emit_t1t(b):
                """T1T[d, o] = sum_c G[d,c] wqa[o,c] — G is symmetric, so
                Gb blocks serve as lhsT directly; no transpose pass."""
                s = st[b]
                Gb, wqa = s["Gb"], s["wqa"]
                T1T = sbp.tile([P, CB, C], BF16, tag="T1b", bufs=1, name=f"TT{b}")
                s["T1T"] = T1T
                for dcb in range(CB):
                    pT1 = psp.tile([P, C], F32, tag="work", bufs=4,
                                   name=f"pT1{b}_{dcb}")
                    for cb in range(CB):
                        nc.tensor.matmul(
                            pT1, Gb[:, cb, ts(dcb, P)], wqa[:, cb, :],
                            start=(cb == 0), stop=(cb == CB - 1),
                        )
                    nc.scalar.copy(T1T[:, dcb, :], pT1)

            def emit_scores(b):
                """scores[o, e] = sum_d T1T[d, o] wka[d, e] + rank-1."""
                s = st[b]
                T1T, wka = s["T1T"], s["wka"]
                cq, sq, ck, rhs1 = s["cq"], s["sq"], s["ck"], s["rhs1"]
                scores = [
                    psp.tile([P, C], F32, tag="scores", bufs=4, name=f"sc{b}_{cb}")
                    for cb in range(CB)
                ]
                s["scores"] = scores
                for ocb in range(CB):
                    for db in range(CB):
                        nc.tensor.matmul(
                            scores[ocb], T1T[:, db, ts(ocb, P)], wka[:, db, :],
                            start=(db == 0), stop=False,
                        )
                    nc.tensor.matmul(
                        scores[ocb], cq[:, ts(ocb, P)], rhs1,
                        start=False, stop=False,
                    )
                    nc.tensor.matmul(
                        scores[ocb], sq[:, ts(ocb, P)], ck,
                        start=False, stop=True,
                    )

            def emit_softmax(b):
                """Max-subtracted exp (x128), row sums -> rinv."""
                s = st[b]
                scores = s["scores"]
                e_sb = sbp.tile([P, CB, C], BF16, tag="e", bufs=1, name=f"e{b}")
                rinv = sbp.tile([P, CB], F32, tag="rinv", bufs=1, name=f"ri{b}")
                rmx = sbp.tile([P, CB], F32, tag="rmx", bufs=1, name=f"rm{b}")
                eb = sbp.tile([P, CB], F32, tag="eb", bufs=1, name=f"eb{b}")
                rsum = sbp.tile([P, CB], F32, tag="rsum", bufs=1, name=f"rs{b}")
                s["e"], s["rinv"] = e_sb, rinv
                for cb in range(CB):
                    nc.vector.reduce_max(
                        rmx[:, cb : cb + 1], scores[cb], axis=AX.X
                    )
                    nc.vector.tensor_scalar(
                        eb[:, cb : cb + 1], rmx[:, cb : cb + 1],
                        -SC2, LN128, op0=OP.mult, op1=OP.add,
                    )
                    nc.scalar.activation(
                        e_sb[:, cb, :], scores[cb], AF.Exp,
                        bias=eb[:, cb : cb + 1], scale=SC2,
                        accum_out=rsum[:, cb : cb + 1],
                    )
                    nc.vector.reciprocal(
                        rinv[:, cb : cb + 1], rsum[:, cb : cb + 1]
                    )

            def emit_m(b):
                """R = (Wo attn)^T = e^T (rinv*Wo^T); r = R^T bvb + bo;
                M'^T[e,o] = I + A[e] * sum_d Wv[d,e] R[d,o]."""
                s = st[b]
                e_sb, rinv, bvb, A_ = s["e"], s["rinv"], s["bvb"], s["A_"]
                wotr = sbp.tile([P, CB, C], BF16, tag="wotr", bufs=2,
                                name=f"wr{b}")
                for cb in range(CB):
                    nc.vector.tensor_scalar_mul(
                        wotr[:, cb, :], wot[:, cb, :], rinv[:, cb : cb + 1]
                    )
                Rb = sbp.tile([P, CB, C], BF16, tag="Rb", bufs=2, name=f"Rb{b}")
                for db in range(CB):
                    pR = psp.tile([P, C], F32, tag="work", bufs=4,
                                  name=f"pR{b}{db}")
                    for cb in range(CB):
                        nc.tensor.matmul(
                            pR, e_sb[:, cb, ts(db, P)], wotr[:, cb, :],
                            start=(cb == 0), stop=(cb == CB - 1),
                        )
                    nc.scalar.copy(Rb[:, db, :], pR)
                # r[o] = sum_d R[d, o] bvb[d] + bo, per-partition layout
                pr = psp.tile([P, CB], F32, tag="work", bufs=4, name=f"pr{b}")
                for ob in range(CB):
                    for db in range(CB):
                        nc.tensor.matmul(
                            pr[:, ob : ob + 1], Rb[:, db, ts(ob, P)],
                            bvb[:, db : db + 1],
                            start=(db == 0), stop=(db == CB - 1),
                        )
                rb = sbp.tile([P, CB], F32, tag="rb", bufs=2, name=f"rv{b}")
                nc.vector.tensor_add(rb, pr, bob)
                s["rb"] = rb
                MtT = sbp.tile([P, CB, C], BF16, tag="MtT", bufs=2,
                               name=f"Mt{b}")
                for eb2 in range(CB):
                    pM = psp.tile([P, C], F32, tag="work", bufs=4,
                                  name=f"pM{b}{eb2}")
                    for db in range(CB):
                        nc.tensor.matmul(
                            pM, wvr[:, db, ts(eb2, P)], Rb[:, db, :],
                            start=(db == 0), stop=(db == CB - 1),
                        )
                    nc.scalar.mul(MtT[:, eb2, :], pM, A_[:, eb2 : eb2 + 1])
                # fold the residual: M' = M + I (diagonal blocks)
                for eb2 in range(CB):
                    nc.vector.tensor_add(
                        MtT[:, eb2, ts(eb2, P)], MtT[:, eb2, ts(eb2, P)],
                        identbf,
                    )
                s["MtT"] = MtT

            def emit_y(b, nsls):
                """Y = M' x + r 1^T for the given pixel slices (bf16 out,
                paired output DMAs alternating sync/gpsimd queues)."""
                s = st[b]
                xbf, MtT, rb = s["xbf"], s["MtT"], s["rb"]
                for nsl in nsls:
                    for ob in range(CB):
                        pf = psp.tile([P, NSL], F32, tag="work", bufs=4,
                                      name=f"pf{b}{nsl}{ob}")
                        for eb2 in range(CB):
                            nc.tensor.matmul(
                                pf, MtT[:, eb2, ts(ob, P)],
                                xbf[:, eb2, ts(nsl, NSL)],
                                start=(eb2 == 0), stop=(eb2 == CB - 1),
                            )
                        yt = sbp.tile([P, NSL], BF16, tag="yt", bufs=4,
                                      name=f"yt{b}{nsl}{ob}")
                        if (nsl * CB + ob) % 2 == 0:
                            nc.vector.tensor_scalar_add(
                                yt, pf, rb[:, ob : ob + 1]
                            )
                        else:
                            nc.scalar.add(yt, pf, rb[:, ob : ob + 1])
                        nc.sync.dma_start(yview[b][:, ob, ts(nsl, NSL)], yt)

            # ---- identbf first (unblocks gram(0)), then x chunks ----
            identbf = sg.tile([P, P], BF16)
            nc.sync.dma_start(identbf, identbf_d[:])
            emit_load(0, NCH)
            # small consts + weights on the gpsimd queue
            gfwd = sg.tile([P, CB, GROUPS], BF16)
            nc.sync.dma_start(gfwd, gfwd_d[:])
            gbwd = sg.tile([GROUPS, CB, P], BF16)
            nc.sync.dma_start(gbwd, gbwd_d[:])
            gam = sg.tile([P, CB], F32)
            nc.sync.dma_start(gam, gamma_d[:].rearrange("(cb p) -> p cb", p=P))
            bet = sg.tile([P, CB], F32)
            nc.sync.dma_start(bet, beta_d[:].rearrange("(cb p) -> p cb", p=P))
            bob = sg.tile([P, CB], F32)
            nc.sync.dma_start(bob, bo_d[:].rearrange("(cb p) -> p cb", p=P))
            bq256r = sg.tile([1, C], F32)
            nc.sync.dma_start(bq256r, bq256_d[:].rearrange("(a c) -> a c", a=1))
            bk256r = sg.tile([1, C], F32)
            nc.sync.dma_start(bk256r, bk256_d[:].rearrange("(a c) -> a c", a=1))
            bvv = sg.tile([1, C], F32)
            nc.sync.dma_start(bvv, bv_d[:].rearrange("(a c) -> a c", a=1))
            eps_g = sg.tile([GROUPS, 1], F32)
            nc.vector.memset(eps_g, EPS)
            # shared pixel-major tile (written per batch) + ones column
            hnT = sg.tile([P, NTH, C + 1], BF16, name="hnT")
            nc.vector.memset(hnT, 1.0)
            ttrj = sg.tile([P, P], F32, name="ttrj")
            # ---- short HAM warm-up while the first x chunks land ----
            zsb = sg.tile([P, NSL], BF16, name="zsb")
            nc.gpsimd.memset(zsb, 0.0)
            pdum = psp.tile([P, NSL], F32, tag="work", bufs=4, name="pdum")
            for i in range(6):
                nc.tensor.matmul(
                    pdum, zsb[:, :P], zsb, start=(i == 0), stop=False
                )
            for cb in range(CB):
                nc.tensor.matmul(
                    pdum, st[0]["xbf"][:, cb, ts(0, P)], zsb,
                    start=False, stop=(cb == CB - 1),
                )
            dsb = sg.tile([1, 1], F32, name="dsb")
            nc.vector.tensor_copy(dsb, pdum[0:1, 0:1])
            dscr = drp.tile([1], F32, name="dscr")
            nc.sync.dma_start(dscr.rearrange("(a c) -> a c", a=1), dsb)
            # ---- weight tiles (first needed by a2/t1t, ~30us in) ----
            wqt16 = sg.tile([P, CB, C], BF16)
            nc.sync.dma_start(wqt16, wqt16_d[:].rearrange("(cb p) o -> p cb o", p=P))
            wkt16 = sg.tile([P, CB, C], BF16)
            nc.sync.dma_start(wkt16, wkt16_d[:].rearrange("(cb p) o -> p cb o", p=P))
            wvt = sg.tile([P, CB, C], BF16)
            nc.sync.dma_start(wvt, wvt_d[:].rearrange("(cb p) o -> p cb o", p=P))
            wvr = sg.tile([P, CB, C], BF16)
            nc.sync.dma_start(wvr, wvr_d[:].rearrange("(cb p) o -> p cb o", p=P))
            wot = sg.tile([P, CB, C], BF16)
            nc.sync.dma_start(wot, wot_d[:].rearrange("(cb p) o -> p cb o", p=P))

            # ---- pipelined schedule (BB=2) ----
            emit_gram(0)           # starts as soon as x chunks land
            emit_stats(0)          # vector, from Gram diag + rowsums
            emit_a2(0)
            emit_load(1, 1)
            emit_t1t(0)
            emit_scores(0)
            emit_softmax(0)
            emit_gram(1)           # PE-heavy; covers softmax(0) latency
            emit_stats(1)
            emit_m(0)
            emit_a2(1)
            emit_y(0, range(0, 4))
            emit_t1t(1)            # mirror-DMA seam covered by m(0)/y(0)
            emit_scores(1)
            emit_y(0, range(4, NS))  # covers softmax(1) latency
            emit_softmax(1)
            emit_m(1)
            emit_y(1, range(NS))

    nc.finalize()
    return nc


def _get_nc():
    if "nc" not in _NC_CACHE:
        _NC_CACHE["nc"] = _build_nc()
    return _NC_CACHE["nc"]


def _make_consts():
    import ml_dtypes

    BF = ml_dtypes.bfloat16
    gfwd = np.zeros((P, CB, GROUPS), np.float32)
    gbwd = np.zeros((GROUPS, CB, P), np.float32)
    for cb in range(CB):
        for p in range(P):
            g = (cb * P + p) // 16
            gfwd[p, cb, g] = 1.0 / (16.0 * N)   # raw sums -> group means
            gbwd[g, cb, p] = 1.0
    return gfwd.astype(BF), gbwd.astype(BF)


def kernel(x, gamma, beta, Wq, bq, Wk, bk, Wv, bv, Wo, bo):
    global LAST_RESULT
    from concourse.bass_utils import run_bass_kernel_spmd

    import ml_dtypes

    BF = ml_dtypes.bfloat16
    x = np.ascontiguousarray(np.asarray(x, np.float32)).reshape(16, C, N)
    xb16 = np.ascontiguousarray(x.astype(BF))
    gfwd, gbwd = _make_consts()
    shared = {
        "wqt16": np.ascontiguousarray(
            (np.asarray(Wq, np.float32).T * 16.0).astype(BF)
        ),
        "wkt16": np.ascontiguousarray(
            (np.asarray(Wk, np.float32).T * 16.0).astype(BF)
        ),
        "wvtb": np.ascontiguousarray(np.asarray(Wv, np.float32).T.astype(BF)),
        "wvrb": np.ascontiguousarray(np.asarray(Wv, np.float32).astype(BF)),
        "wotb": np.ascontiguousarray(np.asarray(Wo, np.float32).T.astype(BF)),
        "bq256": np.ascontiguousarray(np.asarray(bq, np.float32) * 256.0),
        "bk256": np.ascontiguousarray(np.asarray(bk, np.float32) * 256.0),
        "bv": np.ascontiguousarray(np.asarray(bv, np.float32)),
        "bo": np.ascontiguousarray(np.asarray(bo, np.float32)),
        "gamma": np.ascontiguousarray(np.asarray(gamma, np.float32)),
        "beta": np.ascontiguousarray(np.asarray(beta, np.float32)),
        "gfwd": np.ascontiguousarray(gfwd),
        "gbwd": np.ascontiguousarray(gbwd),
        "identbf": np.ascontiguousarray(np.eye(P, dtype=np.float32).astype(BF)),
    }
    in_maps = [
        dict(shared, xsb=np.ascontiguousarray(xb16[BB * i : BB * (i + 1)]))
        for i in range(8)
    ]
    nc = _get_nc()
    import os

    trace = os.environ.get("KERNEL_TRACE") == "1"
    res = run_bass_kernel_spmd(nc, in_maps, core_ids=list(range(8)), trace=trace)
    LAST_RESULT = res
    y = np.concatenate(
        [np.asarray(r["y"], np.float32) for r in res.results], axis=0
    )
    return y.reshape(16, C, 64, 64)


# revision 44
# speedup vs baseline: 1.0345x; 1.0033x over previous
"""AttnBlock (channel attention over 64x64 maps) for Trainium2 — factored
epilogue + transpose-Gram + stats-from-Gram edition.

Data-parallel over batch: 16 batches, 2 per core on 8 NeuronCores.
Per batch [C=512, N=4096], hn = A*x + B (GroupNorm folded to per-channel
affine):

  scores = q^T k with q = Wq hn + bq factorizes through the RAW Gram
    matrix Graw = x @ x^T (C x C):
      scores = (256 Wq A) Graw (256 A Wk)^T / 65536 + rank-1 corrections
    with diag(A) folded into per-batch scaled weights wqa/wka, so the
    pixel-major xT build is a PURE transpose that starts as soon as the
    first x chunks land. The GroupNorm stats come from the Gram itself:
    per-channel sum(x^2) is the Gram diagonal (exact: bf16*bf16 products
    accumulate exactly in fp32) and per-channel sum(x) rides along as a
    ones-column appended to the transposed tiles (blocks 1-3; block 0's
    PSUM bank is full, its rowsum comes from a gpsimd reduce). bn_stats
    and its 22us of serial vector time are gone. The group aggregation
    runs in bf16 matmuls with hi/lo operand splitting (fp32 accuracy at
    bf16 speed).
  The epilogue factors the same way: out = Wo attn v collapses to
      y = M' x + r 1^T,  M' = I + Wo attn Wv diag(A),
      r = Wo attn bvb + bo
    via R = e^T (rinv*Wo^T), M^T = Wv R (A-scaled at eviction, identity
    added), then ONE C*C*N application off the resident bf16 x, evicted
    bf16 with vector/scalar alternation and paired output DMAs spread
    over the sync and gpsimd queues.
Batches are software-pipelined; batch1's Gram covers batch0's softmax;
batch0's Y GEMM is split around batch1's t1t/scores to cover seams.
"""

import sys

if "/opt/trn_rl_repo" not in sys.path:
    sys.path.insert(0, "/opt/trn_rl_repo")

import numpy as np

C = 512          # channels
N = 4096         # pixels (64*64)
BB = 2           # batches per core
P = 128          # partitions
CB = C // P      # 4 channel blocks
NT = N // P      # 32 pixel tiles of 128
NTH = 8          # pixel tiles per hnT chunk
NSL = 512        # pixel slice width (y phase)
NS = N // NSL    # 8 pixel slices
NCH = 4          # x load chunks (batch 0)
GROUPS = 32
EPS = 1e-6
SCALE = float(C) ** -0.5
SC2 = SCALE / 65536.0
LN128 = float(np.log(128.0))

_NC_CACHE = {}
LAST_RESULT = None


def _build_nc():
    import concourse.bacc as bacc
    import concourse.tile as tile
    from concourse import mybir
    from concourse.bass import ts

    F32 = mybir.dt.float32
    BF16 = mybir.dt.bfloat16
    AX = mybir.AxisListType
    AF = mybir.ActivationFunctionType
    OP = mybir.AluOpType

    nc = bacc.Bacc(None, target_bir_lowering=False, num_swdge_queues=4)

    xsb_d = nc.dram_tensor("xsb", [BB, C, N], BF16, kind="ExternalInput")
    wqt16_d = nc.dram_tensor("wqt16", [C, C], BF16, kind="ExternalInput")
    wkt16_d = nc.dram_tensor("wkt16", [C, C], BF16, kind="ExternalInput")
    wvt_d = nc.dram_tensor("wvtb", [C, C], BF16, kind="ExternalInput")
    wvr_d = nc.dram_tensor("wvrb", [C, C], BF16, kind="ExternalInput")
    wot_d = nc.dram_tensor("wotb", [C, C], BF16, kind="ExternalInput")
    bq256_d = nc.dram_tensor("bq256", [C], F32, kind="ExternalInput")
    bk256_d = nc.dram_tensor("bk256", [C], F32, kind="ExternalInput")
    bv_d = nc.dram_tensor("bv", [C], F32, kind="ExternalInput")
    bo_d = nc.dram_tensor("bo", [C], F32, kind="ExternalInput")
    gamma_d = nc.dram_tensor("gamma", [C], F32, kind="ExternalInput")
    beta_d = nc.dram_tensor("beta", [C], F32, kind="ExternalInput")
    gfwd_d = nc.dram_tensor("gfwd", [P, CB, GROUPS], BF16, kind="ExternalInput")
    gbwd_d = nc.dram_tensor("gbwd", [GROUPS, CB, P], BF16, kind="ExternalInput")
    identbf_d = nc.dram_tensor("identbf", [P, P], BF16, kind="ExternalInput")
    y_d = nc.dram_tensor("y", [BB, C, N], BF16, kind="ExternalOutput")

    with tile.TileContext(nc) as tc:
        with (
            tc.tile_pool(name="singles", bufs=1) as sg,
            tc.tile_pool(name="sbp", bufs=1) as sbp,
            tc.tile_pool(name="psp", bufs=1, space="PSUM") as psp,
            tc.tile_pool(name="drp", bufs=1, space="DRAM") as drp,
        ):
            xbview = [xsb_d[b].rearrange("(cb p) n -> p cb n", p=P) for b in range(BB)]
            yview = [y_d[b].rearrange("(ob p) n -> p ob n", p=P) for b in range(BB)]
            st = [dict() for _ in range(BB)]  # per-batch tile state

            def emit_load(b, chunks):
                s = st[b]
                xbf = sbp.tile([P, CB, N], BF16, tag="xbf", bufs=2, name=f"xbf{b}")
                s["xbf"] = xbf
                # chunk-major so early pixel tiles land first
                for ch in range(chunks):
                    for cb in range(CB):
                        nc.sync.dma_start(
                            xbf[:, cb, ts(ch, N // chunks)],
                            xbview[b][:, cb, ts(ch, N // chunks)],
                        )

            def emit_gram(b):
                """xT (pixel-major via PE transpose matmul, plus a ones
                column) -> raw Gram with per-channel rowsums riding along.
                Also kicks the block-0 rowsum reduce on gpsimd."""
                s = st[b]
                xbf = s["xbf"]
                s8 = sbp.tile([P, 8], F32, tag="s8", bufs=2, name=f"s8{b}")
                s1_0 = sbp.tile([P, 1], F32, tag="s1_0", bufs=2, name=f"s10{b}")
                for j in range(8):
                    nc.vector.reduce_sum(
                        s8[:, j : j + 1], xbf[:, 0, ts(j, 512)], AX.X
                    )
                nc.vector.reduce_sum(s1_0, s8, AX.X)
                s["s1_0"] = s1_0
                pG = [
                    psp.tile(
                        [P, C - a * P + (1 if a else 0)], F32,
                        tag="scores", bufs=4, name=f"pG{b}_{a}",
                    )
                    for a in range(CB)
                ]
                s["pG"] = pG
                for half in range(NT // NTH):
                    for ih in range(NTH):
                        i = half * NTH + ih
                        pT = psp.tile([P, C], F32, tag="work", bufs=4,
                                      name=f"pT{b}_{i}")
                        for cb in range(CB):
                            nc.tensor.matmul(
                                pT[:, ts(cb, P)], xbf[:, cb, ts(i, P)],
                                identbf, start=True, stop=True,
                            )
                        if ih % 2 == 0:
                            nc.scalar.copy(hnT[:, ih, :C], pT)
                        else:
                            nc.vector.tensor_copy(hnT[:, ih, :C], pT)
                    for ih in range(NTH):
                        i = half * NTH + ih
                        for a in range(CB):
                            nc.tensor.matmul(
                                pG[a], hnT[:, ih, ts(a, P)],
                                hnT[:, ih, a * P : C + (1 if a else 0)],
                                start=(i == 0), stop=(i == NT - 1),
                            )
                Gb = sbp.tile([P, CB, C], BF16, tag="Gb", bufs=1, name=f"Gb{b}")
                for a in range(CB):
                    nc.scalar.copy(Gb[:, a, a * P :], pG[a][:, : C - a * P])
                # mirror the 6 sub-diagonal blocks: G[b,a] = G[a,b]^T
                for a in range(CB):
                    for bb2 in range(a + 1, CB):
                        nc.sync.dma_start(
                            Gb[:, bb2, ts(a, P)],
                            Gb[:, a, ts(bb2, P)],
                            transpose=True,
                        )
                s["Gb"] = Gb

            def emit_stats(b):
                """Per-channel [sum(x), sum(x^2)] -> t_hi/t_lo (bf16 split)
                from the Gram diagonal + ones-column rowsums."""
                s = st[b]
                pG, s1_0 = s["pG"], s["s1_0"]
                t32 = sbp.tile([P, CB, 2], F32, tag="t32", bufs=2, name=f"t{b}")
                nc.vector.tensor_copy(t32[:, 0, 0:1], s1_0)
                for a in range(1, CB):
                    w = C - a * P
                    nc.vector.tensor_copy(t32[:, a, 0:1], pG[a][:, w : w + 1])
                for a in range(CB):
                    nc.vector.tensor_tensor(
                        ttrj, pG[a][:, :P], identbf, op=OP.mult
                    )
                    nc.vector.reduce_sum(t32[:, a, 1:2], ttrj, AX.X)
                t_hi = sbp.tile([P, CB, 2], BF16, tag="t_hi", bufs=2,
                                name=f"th{b}")
                t_lo = sbp.tile([P, CB, 2], BF16, tag="t_lo", bufs=2,
                                name=f"tl{b}")
                nc.vector.tensor_copy(t_hi, t32)
                nc.vector.tensor_tensor(t_lo, t32, t_hi, op=OP.subtract)
                s["t32"], s["t_hi"], s["t_lo"] = t32, t_hi, t_lo

            def emit_a2(b):
                """Group aggregation -> A, B; A-scaled wq/wk; biases;
                scores rank-1 correction vectors. bf16 GEMMs with hi/lo
                operand splitting."""
                s = st[b]
                t32, t_hi, t_lo = s["t32"], s["t_hi"], s["t_lo"]
                pg = psp.tile([GROUPS, 2], F32, tag="work", bufs=4, name=f"pg{b}")
                for cb in range(CB):
                    nc.tensor.matmul(
                        pg, gfwd[:, cb, :], t_hi[:, cb, :],
                        start=(cb == 0), stop=False,
                    )
                    nc.tensor.matmul(
                        pg, gfwd[:, cb, :], t_lo[:, cb, :],
                        start=False, stop=(cb == CB - 1),
                    )
                gs = sbp.tile([GROUPS, 2], F32, tag="gs", bufs=2, name=f"gs{b}")
                pgs = sbp.tile([GROUPS, 2], F32, tag="pgs", bufs=2, name=f"pgs{b}")
                nc.vector.tensor_copy(pgs, pg)
                vtmp = sbp.tile([GROUPS, 1], F32, tag="vtmp", bufs=2, name=f"vt{b}")
                nc.vector.tensor_mul(vtmp, pgs[:, 0:1], pgs[:, 0:1])
                nc.vector.tensor_tensor(vtmp, pgs[:, 1:2], vtmp, op=OP.subtract)
                nc.vector.tensor_copy(gs[:, 0:1], pgs[:, 0:1])
                nc.scalar.activation(gs[:, 1:2], vtmp, AF.Sqrt, bias=eps_g)
                nc.vector.reciprocal(gs[:, 1:2], gs[:, 1:2])
                gs_hi = sbp.tile([GROUPS, 2], BF16, tag="gs_hi", bufs=2,
                                 name=f"gh{b}")
                gs_lo = sbp.tile([GROUPS, 2], BF16, tag="gs_lo", bufs=2,
                                 name=f"gl{b}")
                nc.vector.tensor_copy(gs_hi, gs)
                nc.vector.tensor_tensor(gs_lo, gs, gs_hi, op=OP.subtract)

                cst = sbp.tile([P, CB, 2], F32, tag="cst", bufs=2, name=f"cs{b}")
                for cb in range(CB):
                    pc = psp.tile([P, 2], F32, tag="work", bufs=4, name=f"pc{b}_{cb}")
                    nc.tensor.matmul(pc, gbwd[:, cb, :], gs_hi,
                                     start=True, stop=False)
                    nc.tensor.matmul(pc, gbwd[:, cb, :], gs_lo,
                                     start=False, stop=True)
                    nc.vector.tensor_copy(cst[:, cb, :], pc)

                A_ = sbp.tile([P, CB], F32, tag="A_", bufs=2, name=f"A{b}")
                # brs packs (B, rs) as two lhsT columns for the matvec GEMMs
                brs = sbp.tile([P, CB, 2], BF16, tag="brs", bufs=2, name=f"brs{b}")
                tmpB = sbp.tile([P, CB], F32, tag="tmpB", bufs=2, name=f"tB{b}")
                nc.vector.tensor_mul(A_, cst[:, :, 1], gam)
                nc.vector.tensor_mul(tmpB, cst[:, :, 0], A_)
                nc.vector.tensor_tensor(brs[:, :, 0], bet, tmpB, op=OP.subtract)
                s["A_"] = A_

                # per-batch A-scaled weights for the Gram->scores GEMMs
                A16 = sbp.tile([P, CB], F32, tag="A16", bufs=2, name=f"A16{b}")
                nc.vector.tensor_scalar_mul(A16, A_, 16.0)
                wqa = sbp.tile([P, CB, C], BF16, tag="wqa", bufs=2, name=f"wqa{b}")
                wka = sbp.tile([P, CB, C], BF16, tag="wka", bufs=2, name=f"wka{b}")
                for cb in range(CB):
                    nc.vector.tensor_scalar_mul(
                        wqa[:, cb, :], wqt16[:, cb, :], A16[:, cb : cb + 1]
                    )
                    nc.vector.tensor_scalar_mul(
                        wka[:, cb, :], wkt16[:, cb, :], A16[:, cb : cb + 1]
                    )
                s["wqa"], s["wka"] = wqa, wka

                # v bias: bvb = bv + Wv@B, via DRAM round-trip to [P, CB]
                pb = psp.tile([1, C], F32, tag="work", bufs=4, name=f"pbv{b}")
                for cb in range(CB):
                    nc.tensor.matmul(
                        pb, brs[:, cb, 0:1], wvt[:, cb, :],
                        start=(cb == 0), stop=(cb == CB - 1),
                    )
                bvrow = sbp.tile([1, C], F32, tag="bvrow", bufs=2, name=f"bvr{b}")
                nc.vector.tensor_add(bvrow, pb, bvv)
                scr = drp.tile([C], F32, name=f"scrv{b}")
                nc.sync.dma_start(scr.rearrange("(a c) -> a c", a=1), bvrow)
                bvb = sbp.tile([P, CB], F32, tag="bvb", bufs=2, name=f"bvb{b}")
                nc.sync.dma_start(bvb, scr.rearrange("(cb p) -> p cb", p=P))
                bvb16 = sbp.tile([P, CB], BF16, tag="bvb16", bufs=2,
                                 name=f"bvb16{b}")
                nc.vector.tensor_copy(bvb16, bvb)
                s["bvb"] = bvb16

                # scores rank-1 vectors (x256 scale):
                #   cq256 = 256*(Wq@B + bq), sq256 = 256*(Wq@rs),
                #   rs = A*sum(x) (raw sums, so 16.0 not 16N)
                rsf = sbp.tile([P, CB], F32, tag="rsf", bufs=2, name=f"rsf{b}")
                nc.vector.tensor_mul(rsf, A_, t32[:, :, 0])
                nc.vector.tensor_scalar_mul(brs[:, :, 1], rsf, 16.0)
                rows = {}
                for nm, wt, brow in (("q", wqt16, bq256r), ("k", wkt16, bk256r)):
                    pc1 = psp.tile([1, C], F32, tag="work", bufs=4,
                                   name=f"pc1{b}{nm}")
                    for cb in range(CB):
                        nc.tensor.matmul(
                            pc1, brs[:, cb, 0:1], wt[:, cb, :],
                            start=(cb == 0), stop=(cb == CB - 1),
                        )
                    crow = sbp.tile([1, C], BF16, tag=f"c{nm}row", bufs=2,
                                    name=f"c{nm}{b}")
                    tmpr = sbp.tile([1, C], F32, tag="tmpr", bufs=2,
                                    name=f"tr{b}{nm}")
                    nc.vector.tensor_scalar_mul(tmpr, pc1, 16.0)
                    nc.vector.tensor_add(crow, tmpr, brow)
                    rows[f"c{nm}"] = crow
                    ps1 = psp.tile([1, C], F32, tag="work", bufs=4,
                                   name=f"ps1{b}{nm}")
                    for cb in range(CB):
                        nc.tensor.matmul(
                            ps1, brs[:, cb, 1:2], wt[:, cb, :],
                            start=(cb == 0), stop=(cb == CB - 1),
                        )
                    srow = sbp.tile([1, C], BF16, tag=f"s{nm}row", bufs=2,
                                    name=f"s{nm}{b}")
                    nc.vector.tensor_copy(srow, ps1)
                    rows[f"s{nm}"] = srow
                rhs1 = sbp.tile([1, C], BF16, tag="rhs1", bufs=2, name=f"rh{b}")
                nc.vector.tensor_scalar_mul(rhs1, rows["ck"], float(N))
                nc.vector.tensor_add(rhs1, rhs1, rows["sk"])
                s["cq"], s["sq"], s["ck"] = rows["cq"], rows["sq"], rows["ck"]
                s["rhs1"] = rhs1

            def emit_t1t(b):
                """T1T[d, o] = sum_c G[d,c] wqa[o,c] — G is symmetric, so
                Gb blocks serve as lhsT directly; no transpose pass."""
                s = st[b]
                Gb, wqa = s["Gb"], s["wqa"]
                T1T = sbp.tile([P, CB, C], BF16, tag="T1b", bufs=1, name=f"TT{b}")
                s["T1T"] = T1T
                for dcb in range(CB):
                    pT1 = psp.tile([P, C], F32, tag="work", bufs=4,
                                   name=f"pT1{b}_{dcb}")
                    for cb in range(CB):
                        nc.tensor.matmul(
                            pT1, Gb[:, cb, ts(dcb, P)], wqa[:, cb, :],
                            start=(cb == 0), stop=(cb == CB - 1),
                        )
                    nc.scalar.copy(T1T[:, dcb, :], pT1)

            def emit_scores(b):
                """scores[o, e] = sum_d T1T[d, o] wka[d, e] + rank-1."""
                s = st[b]
                T1T, wka = s["T1T"], s["wka"]
                cq, sq, ck, rhs1 = s["cq"], s["sq"], s["ck"], s["rhs1"]
                scores = [
                    psp.tile([P, C], F32, tag="scores", bufs=4, name=f"sc{b}_{cb}")
                    for cb in range(CB)
                ]
                s["scores"] = scores
                for ocb in range(CB):
                    for db in range(CB):
                        nc.tensor.matmul(
                            scores[ocb], T1T[:, db, ts(ocb, P)], wka[:, db, :],
                            start=(db == 0), stop=False,
                        )
                    nc.tensor.matmul(
                        scores[ocb], cq[:, ts(ocb, P)], rhs1,
                        start=False, stop=False,
                    )
                    nc.tensor.matmul(
                        scores[ocb], sq[:, ts(ocb, P)], ck,
                        start=False, stop=True,
                    )

            def emit_softmax(b):
                """Max-subtracted exp (x128), row sums -> rinv."""
                s = st[b]
                scores = s["scores"]
                e_sb = sbp.tile([P, CB, C], BF16, tag="e", bufs=1, name=f"e{b}")
                rinv = sbp.tile([P, CB], F32, tag="rinv", bufs=1, name=f"ri{b}")
                rmx = sbp.tile([P, CB], F32, tag="rmx", bufs=1, name=f"rm{b}")
                eb = sbp.tile([P, CB], F32, tag="eb", bufs=1, name=f"eb{b}")
                rsum = sbp.tile([P, CB], F32, tag="rsum", bufs=1, name=f"rs{b}")
                s["e"], s["rinv"] = e_sb, rinv
                for cb in range(CB):
                    nc.vector.reduce_max(
                        rmx[:, cb : cb + 1], scores[cb], axis=AX.X
                    )
                    nc.vector.tensor_scalar(
                        eb[:, cb : cb + 1], rmx[:, cb : cb + 1],
                        -SC2, LN128, op0=OP.mult, op1=OP.add,
                    )
                    nc.scalar.activation(
                        e_sb[:, cb, :], scores[cb], AF.Exp,
                        bias=eb[:, cb : cb + 1], scale=SC2,
                        accum_out=rsum[:, cb : cb + 1],
                    )
                    nc.vector.reciprocal(
                        rinv[:, cb : cb + 1], rsum[:, cb : cb + 1]
                    )

            def emit_m(b):
                """R = (Wo attn)^T = e^T (rinv*Wo^T); r = R^T bvb + bo;
                M'^T[e,o] = I + A[e] * sum_d Wv[d,e] R[d,o]."""
                s = st[b]
                e_sb, rinv, bvb, A_ = s["e"], s["rinv"], s["bvb"], s["A_"]
                wotr = sbp.tile([P, CB, C], BF16, tag="wotr", bufs=2,
                                name=f"wr{b}")
                for cb in range(CB):
                    nc.vector.tensor_scalar_mul(
                        wotr[:, cb, :], wot[:, cb, :], rinv[:, cb : cb + 1]
                    )
                Rb = sbp.tile([P, CB, C], BF16, tag="Rb", bufs=2, name=f"Rb{b}")
                for db in range(CB):
                    pR = psp.tile([P, C], F32, tag="work", bufs=4,
                                  name=f"pR{b}{db}")
                    for cb in range(CB):
                        nc.tensor.matmul(
                            pR, e_sb[:, cb, ts(db, P)], wotr[:, cb, :],
                            start=(cb == 0), stop=(cb == CB - 1),
                        )
                    nc.scalar.copy(Rb[:, db, :], pR)
                # r[o] = sum_d R[d, o] bvb[d] + bo, per-partition layout
                pr = psp.tile([P, CB], F32, tag="work", bufs=4, name=f"pr{b}")
                for ob in range(CB):
                    for db in range(CB):
                        nc.tensor.matmul(
                            pr[:, ob : ob + 1], Rb[:, db, ts(ob, P)],
                            bvb[:, db : db + 1],
                            start=(db == 0), stop=(db == CB - 1),
                        )
                rb = sbp.tile([P, CB], F32, tag="rb", bufs=2, name=f"rv{b}")
                nc.vector.tensor_add(rb, pr, bob)
                s["rb"] = rb
                MtT = sbp.tile([P, CB, C], BF16, tag="MtT", bufs=2,
                               name=f"Mt{b}")
                for eb2 in range(CB):
                    pM = psp.tile([P, C], F32, tag="work", bufs=4,
                                  name=f"pM{b}{eb2}")
                    for db in range(CB):
                        nc.tensor.matmul(
                            pM, wvr[:, db, ts(eb2, P)], Rb[:, db, :],
                            start=(db == 0), stop=(db == CB - 1),
                        )
                    nc.scalar.mul(MtT[:, eb2, :], pM, A_[:, eb2 : eb2 + 1])
                # fold the residual: M' = M + I (diagonal blocks)
                for eb2 in range(CB):
                    nc.vector.tensor_add(
                        MtT[:, eb2, ts(eb2, P)], MtT[:, eb2, ts(eb2, P)],
                        identbf,
                    )
                s["MtT"] = MtT

            def emit_y(b, nsls):
                """Y = M' x + r 1^T for the given pixel slices (bf16 out,
                paired output DMAs alternating sync/gpsimd queues)."""
                s = st[b]
                xbf, MtT, rb = s["xbf"], s["MtT"], s["rb"]
                for nsl in nsls:
                    for ob in range(CB):
                        pf = psp.tile([P, NSL], F32, tag="work", bufs=4,
                                      name=f"pf{b}{nsl}{ob}")
                        for eb2 in range(CB):
                            nc.tensor.matmul(
                                pf, MtT[:, eb2, ts(ob, P)],
                                xbf[:, eb2, ts(nsl, NSL)],
                                start=(eb2 == 0), stop=(eb2 == CB - 1),
                            )
                        yt = sbp.tile([P, NSL], BF16, tag="yt", bufs=4,
                                      name=f"yt{b}{nsl}{ob}")
                        if (nsl * CB + ob) % 2 == 0:
                            nc.vector.tensor_scalar_add(
                                yt, pf, rb[:, ob : ob + 1]
                            )
                        else:
                            nc.scalar.add(yt, pf, rb[:, ob : ob + 1])
                        nc.sync.dma_start(yview[b][:, ob, ts(nsl, NSL)], yt)

            # ---- identbf first (unblocks gram(0)), then x chunks ----
            identbf = sg.tile([P, P], BF16)
            nc.sync.dma_start(identbf, identbf_d[:])
            emit_load(0, NCH)
            # small consts + weights on the gpsimd queue
            gfwd = sg.tile([P, CB, GROUPS], BF16)
            nc.sync.dma_start(gfwd, gfwd_d[:])
            gbwd = sg.tile([GROUPS, CB, P], BF16)
            nc.sync.dma_start(gbwd, gbwd_d[:])
            gam = sg.tile([P, CB], F32)
            nc.sync.dma_start(gam, gamma_d[:].rearrange("(cb p) -> p cb", p=P))
            bet = sg.tile([P, CB], F32)
            nc.sync.dma_start(bet, beta_d[:].rearrange("(cb p) -> p cb", p=P))
            bob = sg.tile([P, CB], F32)
            nc.sync.dma_start(bob, bo_d[:].rearrange("(cb p) -> p cb", p=P))
            bq256r = sg.tile([1, C], F32)
            nc.sync.dma_start(bq256r, bq256_d[:].rearrange("(a c) -> a c", a=1))
            bk256r = sg.tile([1, C], F32)
            nc.sync.dma_start(bk256r, bk256_d[:].rearrange("(a c) -> a c", a=1))
            bvv = sg.tile([1, C], F32)
            nc.sync.dma_start(bvv, bv_d[:].rearrange("(a c) -> a c", a=1))
            eps_g = sg.tile([GROUPS, 1], F32)
            nc.vector.memset(eps_g, EPS)
            # shared pixel-major tile (written per batch) + ones column
            hnT = sg.tile([P, NTH, C + 1], BF16, name="hnT")
            nc.vector.memset(hnT, 1.0)
            ttrj = sg.tile([P, P], F32, name="ttrj")
            # ---- short HAM warm-up while the first x chunks land ----
            zsb = sg.tile([P, NSL], BF16, name="zsb")
            nc.gpsimd.memset(zsb, 0.0)
            pdum = psp.tile([P, NSL], F32, tag="work", bufs=4, name="pdum")
            for i in range(6):
                nc.tensor.matmul(
                    pdum, zsb[:, :P], zsb, start=(i == 0), stop=False
                )
            for cb in range(CB):
                nc.tensor.matmul(
                    pdum, st[0]["xbf"][:, cb, ts(0, P)], zsb,
                    start=False, stop=(cb == CB - 1),
                )
            dsb = sg.tile([1, 1], F32, name="dsb")
            nc.vector.tensor_copy(dsb, pdum[0:1, 0:1])
            dscr = drp.tile([1], F32, name="dscr")
            nc.sync.dma_start(dscr.rearrange("(a c) -> a c", a=1), dsb)
            # ---- weight tiles (first needed by a2/t1t, ~30us in) ----
            wqt16 = sg.tile([P, CB, C], BF16)
            nc.sync.dma_start(wqt16, wqt16_d[:].rearrange("(cb p) o -> p cb o", p=P))
            wkt16 = sg.tile([P, CB, C], BF16)
            nc.sync.dma_start(wkt16, wkt16_d[:].rearrange("(cb p) o -> p cb o", p=P))
            wvt = sg.tile([P, CB, C], BF16)
            nc.sync.dma_start(wvt, wvt_d[:].rearrange("(cb p) o -> p cb o", p=P))
            wvr = sg.tile([P, CB, C], BF16)
            nc.sync.dma_start(wvr, wvr_d[:].rearrange("(cb p) o -> p cb o", p=P))
            wot = sg.tile([P, CB, C], BF16)
            nc.sync.dma_start(wot, wot_d[:].rearrange("(cb p) o -> p cb o", p=P))

            # ---- pipelined schedule (BB=2) ----
            emit_gram(0)           # starts as soon as x chunks land
            emit_stats(0)          # vector, from Gram diag + rowsums
            emit_a2(0)
            emit_load(1, 1)
            emit_t1t(0)
            emit_scores(0)
            emit_softmax(0)
            emit_gram(1)           # PE-heavy; covers softmax(0) latency
            emit_stats(1)
            emit_m(0)
            emit_a2(1)
            emit_y(0, range(0, 4))
            emit_t1t(1)            # mirror-DMA seam covered by m(0)/y(0)
            emit_scores(1)
            emit_y(0, range(4, NS))  # covers softmax(1) latency
            emit_softmax(1)
            emit_m(1)
            emit_y(1, range(NS))

    nc.finalize()
    return nc


def _get_nc():
    if "nc" not in _NC_CACHE:
        _NC_CACHE["nc"] = _build_nc()
    return _NC_CACHE["nc"]


def _make_consts():
    import ml_dtypes

    BF = ml_dtypes.bfloat16
    gfwd = np.zeros((P, CB, GROUPS), np.float32)
    gbwd = np.zeros((GROUPS, CB, P), np.float32)
    for cb in range(CB):
        for p in range(P):
            g = (cb * P + p) // 16
            gfwd[p, cb, g] = 1.0 / (16.0 * N)   # raw sums -> group means
            gbwd[g, cb, p] = 1.0
    return gfwd.astype(BF), gbwd.astype(BF)


def kernel(x, gamma, beta, Wq, bq, Wk, bk, Wv, bv, Wo, bo):
    global LAST_RESULT
    from concourse.bass_utils import run_bass_kernel_spmd

    import ml_dtypes

    BF = ml_dtypes.bfloat16
    x = np.ascontiguousarray(np.asarray(x, np.float32)).reshape(16, C, N)
    xb16 = np.ascontiguousarray(x.astype(BF))
    gfwd, gbwd = _make_consts()
    shared = {
        "wqt16": np.ascontiguousarray(
            (np.asarray(Wq, np.float32).T * 16.0).astype(BF)
        ),
        "wkt16": np.ascontiguousarray(
            (np.asarray(Wk, np.float32).T * 16.0).astype(BF)
        ),
        "wvtb": np.ascontiguousarray(np.asarray(Wv, np.float32).T.astype(BF)),
        "wvrb": np.ascontiguousarray(np.asarray(Wv, np.float32).astype(BF)),
        "wotb": np.ascontiguousarray(np.asarray(Wo, np.float32).T.astype(BF)),
        "bq256": np.ascontiguousarray(np.asarray(bq, np.float32) * 256.0),
        "bk256": np.ascontiguousarray(np.asarray(bk, np.float32) * 256.0),
        "bv": np.ascontiguousarray(np.asarray(bv, np.float32)),
        "bo": np.ascontiguousarray(np.asarray(bo, np.float32)),
        "gamma": np.ascontiguousarray(np.asarray(gamma, np.float32)),
        "beta": np.ascontiguousarray(np.asarray(beta, np.float32)),
        "gfwd": np.ascontiguousarray(gfwd),
        "gbwd": np.ascontiguousarray(gbwd),
        "identbf": np.ascontiguousarray(np.eye(P, dtype=np.float32).astype(BF)),
    }
    in_maps = [
        dict(shared, xsb=np.ascontiguousarray(xb16[BB * i : BB * (i + 1)]))
        for i in range(8)
    ]
    nc = _get_nc()
    import os

    trace = os.environ.get("KERNEL_TRACE") == "1"
    res = run_bass_kernel_spmd(nc, in_maps, core_ids=list(range(8)), trace=trace)
    LAST_RESULT = res
    y = np.concatenate(
        [np.asarray(r["y"], np.float32) for r in res.results], axis=0
    )
    return y.reshape(16, C, 64, 64)
